# revision 20
# baseline (speedup 1.0000x reference)
"""Sliding-window causal attention (B=2,T=2048,C=1024,H=16,D=64,W=256) on 8 trn2 cores.

Sharding: core c = (batch b = c//4, head-group g = c%4 of 4 heads).
Each core computes q/k/v projections for its 4 heads on its batch, windowed
attention per head, and a partial output projection (its 256 channels of the
contraction); host sums the 4 partials per batch.

Layout strategy (no on-chip transposes):
  - host ships xT = x[b].T  [C, T] bf16  (lhsT/rhs for projections)
  - qT, kT computed transposed [256, T] (channels on partitions)
  - v computed natural [T, 256] (tokens on partitions) -> PV lhsT
  - scores computed transposed S^T[k, q] so softmax sum comes from a
    ones-row folded into the PV matmul; per-q reciprocal broadcast along
    partitions via a rank-2 selector matmul on the PE (no DMA bounce).

Schedule (keeps the PE dense so HAM stays at 2.4GHz):
  - zero-weight warmup matmuls folded into the first projection group's
    accumulation, paced by DMA-chunk dependencies
  - m0 q/k projections run kt-major (contraction-major) so each xT quarter
    unlocks a full stripe of matmuls as it lands
  - m1 projections + v projection interleave into the mp0 attention loop,
    out-projection interleaves into the mp1 attention loop
  - PV uses clipped wide-N matmuls (6 per head-block) relying on per-element
    PSUM has_written accumulate semantics
"""

import os
import sys

sys.path.insert(0, "/opt/trn_rl_repo")

import numpy as np
import ml_dtypes

import concourse.bass as bass
import concourse.tile as tile
from concourse import bacc
from concourse import mybir
from concourse.bass import ds, ts

BF16 = ml_dtypes.bfloat16

B, T, C = 2, 2048, 1024
H, W, D = 16, 256, 64
HPC = 4          # heads per core
CL = HPC * D     # 256 local channels per core
NKT = C // 128   # 8 contraction tiles for projections
NT = T // 128    # 16 token tiles
SCALE = 0.125    # 1/sqrt(D)
F32 = mybir.dt.float32
BF = mybir.dt.bfloat16
QB = 512         # normalize/outproj granularity along q (4 query tiles)


def build_program():
    nc = bacc.Bacc("TRN2", target_bir_lowering=False, debug=False)

    n_warm = int(os.environ.get("ATTN_WARMUP_MMS", "16"))

    # consolidated inputs: SBUF-row-contiguous so DMA descriptors are 4-32KB
    xT_d = nc.dram_tensor("xTt", [128, NKT * T], BF, kind="ExternalInput")
    w_d = nc.dram_tensor("wt", [128, 3 * NKT * CL], BF, kind="ExternalInput")
    wo_d = nc.dram_tensor("wot", [128, 2 * C], BF, kind="ExternalInput")
    mask_d = nc.dram_tensor("maskt", [128, 512], BF, kind="ExternalInput")
    y_d = nc.dram_tensor("y", [T, C], BF, kind="ExternalOutput")

    with tile.TileContext(nc) as tc:
        with (
            tc.tile_pool(name="const", bufs=1) as constp,
            tc.tile_pool(name="acts", bufs=1) as actsp,
            tc.tile_pool(name="epool", bufs=8) as ep,
            tc.tile_pool(name="small", bufs=2) as smallp,
            tc.tile_pool(name="ysb", bufs=3) as yp,
            tc.tile_pool(name="ps", bufs=1, space="PSUM") as psp,
            tc.tile_pool(name="dramp", bufs=2, space="DRAM") as dramp,
        ):
            norm_mode_early = os.environ.get("ATTN_NORM", "gps")
            if norm_mode_early == "gps":
                from concourse import library_config

                nc.gpsimd.load_library(library_config.attn)

            # ---- constants built on-chip (no DMA dependency) ----
            z_sb = constp.tile([128, 512], BF, tag="zeros", name="z_sb")
            nc.gpsimd.memset(z_sb[:], 0.0)

            # ---- static SBUF tiles + loads, in need-order ----
            mask_all = constp.tile([128, 512], BF, tag="maskall", name="mask_all")
            w_all = constp.tile([128, 3 * NKT * CL], BF, tag="wall", name="w_all")
            wo_all = constp.tile([128, 2 * C], BF, tag="woall", name="wo_all")
            xT_all = constp.tile([128, NKT * T], BF, tag="xTall", name="xT_all")

            nc.sync.dma_start(mask_all[:], mask_d[:])
            # wq+wk first (gate the m0 projections), wv after xT, wo last
            nc.sync.dma_start(w_all[:, 0 : 2 * NKT * CL], w_d[:, 0 : 2 * NKT * CL])
            for q4 in range(4):
                nc.sync.dma_start(
                    xT_all[:, ds(q4 * 2 * T, 2 * T)], xT_d[:, ds(q4 * 2 * T, 2 * T)]
                )
            nc.sync.dma_start(
                w_all[:, ds(2 * NKT * CL, NKT * CL)], w_d[:, ds(2 * NKT * CL, NKT * CL)]
            )
            nc.sync.dma_start(wo_all[:], wo_d[:])

            xT_sb = [xT_all[:, ds(i * T, T)] for i in range(NKT)]
            wq_sb = [w_all[:, ds((0 * NKT + i) * CL, CL)] for i in range(NKT)]
            wk_sb = [w_all[:, ds((1 * NKT + i) * CL, CL)] for i in range(NKT)]
            wv_sb = [w_all[:, ds((2 * NKT + i) * CL, CL)] for i in range(NKT)]
            wo_sb = [wo_all[:, ds(j * C, C)] for j in range(2)]
            # host layout [maskd|masks|maskd|masks]
            maskf_v = mask_all.rearrange("p (b s x) -> p b s x", b=2, s=2)
            maskd_v = maskf_v[:, :, 0, :]

            # persistent activations
            qT_sb = [actsp.tile([128, T], BF, tag=f"qT{m}", name=f"qT{m}") for m in range(2)]
            kT_sb = [actsp.tile([128, T], BF, tag=f"kT{m}", name=f"kT{m}") for m in range(2)]
            aT_sb = [actsp.tile([128, T], BF, tag=f"aT{m}", name=f"aT{m}") for m in range(2)]
            # v natural layout, per token-tile: [v_h0(64)|1|v_h1(64)|1|...] = 260 cols
            v_sb = [actsp.tile([128, 4 * 65], BF, tag=f"v{t}", name=f"v{t}") for t in range(NT)]
            for t in range(NT):
                vv = v_sb[t].rearrange("p (h c) -> p h c", h=4)
                nc.gpsimd.memset(vv[:, :, 64:65], 1.0)

            # PSUM budget (8 banks): 'sc' 1x[128,1024] = 2, 'proj' 4x[128,512]
            # = 4, 'pv' 2x[128,512] = 2. P0 uses all three tags for its 8
            # kt-major groups; attention reuses them (ring order = evac order).
            def proj_ps(name):
                return psp.tile([128, 512], F32, tag="proj", bufs=4, name=name)

            def pv_ps(name, shape=(65, QB)):
                return psp.tile(
                    list(shape), F32, tag="pv", bufs=2,
                    padded_shape=[128, QB], name=name,
                )

            # ---- P0: warmup + kt-major m0 q/k projections ----
            grp_defs = [("q", 0), ("k", 0), ("q", 1), ("k", 1), ("q", 2), ("q", 3)]
            grp_ps = [proj_ps(f"ps_{pr}{n}") for pr, n in grp_defs[:4]] + [
                pv_ps(f"ps_{pr}{n}", shape=(128, 512)) for pr, n in grp_defs[4:]
            ]
            kn23 = psp.tile([128, 1024], F32, tag="sc", bufs=1, name="ps_kn23")

            # zero-weight warmups accumulate into group 0's psum: they add 0 and
            # keep the PE active (HAM at 2.4GHz) through the runtime preamble +
            # input DMA window. Later warmups read just-DMA'd tiles to pace out.
            if n_warm:
                warm_rhs = (
                    [z_sb[:]] * max(0, n_warm - 8)
                    + [mask_all[:, 0:512]] * min(4, n_warm)
                    + [w_all[:, 0:512]] * min(4, max(0, n_warm - 4))
                )
                for i, rhs in enumerate(warm_rhs):
                    nc.tensor.matmul(
                        grp_ps[0][:], lhsT=z_sb[:, 0:128], rhs=rhs,
                        start=(i == 0), stop=False,
                    )

            for kt in range(NKT):
                for gi, (pr, n) in enumerate(grp_defs):
                    wsel = wq_sb if pr == "q" else wk_sb
                    nc.tensor.matmul(
                        grp_ps[gi][:],
                        lhsT=wsel[kt][:, 0:128],
                        rhs=xT_sb[kt][:, ts(n, 512)],
                        start=(kt == 0 and not (gi == 0 and n_warm)),
                        stop=(kt == NKT - 1),
                    )
                for half, n in enumerate((2, 3)):
                    nc.tensor.matmul(
                        kn23[:, ts(half, 512)],
                        lhsT=wk_sb[kt][:, 0:128],
                        rhs=xT_sb[kt][:, ts(n, 512)],
                        start=(kt == 0),
                        stop=(kt == NKT - 1),
                    )

            # evacuations: kn2/kn3 first (frees the scores slot), then in the
            # 'proj' ring order so attention-phase ring allocations gate cleanly
            nc.scalar.copy(kT_sb[0][:, ts(2, 512)], kn23[:, 0:512])
            nc.vector.tensor_copy(kT_sb[0][:, ts(3, 512)], kn23[:, 512:1024])
            for gi, (pr, n) in enumerate(grp_defs):
                dstT = qT_sb if pr == "q" else kT_sb
                if gi % 2 == 0:
                    nc.scalar.copy(dstT[0][:, ts(n, 512)], grp_ps[gi][:])
                else:
                    nc.vector.tensor_copy(dstT[0][:, ts(n, 512)], grp_ps[gi][:])

            # ---- attention-phase building blocks ----
            # m1 projection groups: fillers, issued as 4-MM half-chunks
            m1_defs = [(pr, n) for n in range(4) for pr in ("q", "k")]
            m1_state = {}

            def m1_chunk(ci):
                gi = ci // 2
                if gi >= len(m1_defs):
                    return
                pr, n = m1_defs[gi]
                if ci % 2 == 0:
                    m1_state["ps"] = proj_ps("ps_m1")
                pse = m1_state["ps"]
                wsel = wq_sb if pr == "q" else wk_sb
                for kt in range(4 * (ci % 2), 4 * (ci % 2) + 4):
                    nc.tensor.matmul(
                        pse[:],
                        lhsT=wsel[kt][:, 128:256],
                        rhs=xT_sb[kt][:, ts(n, 512)],
                        start=(kt == 0),
                        stop=(kt == NKT - 1),
                    )
                if ci % 2 == 1:
                    dstT = qT_sb if pr == "q" else kT_sb
                    if gi % 2 == 0:
                        nc.scalar.copy(dstT[1][:, ts(n, 512)], pse[:])
                    else:
                        nc.vector.tensor_copy(dstT[1][:, ts(n, 512)], pse[:])

            def v_tile(t):
                psv = psp.tile([128, CL], F32, tag="proj", bufs=4, name="ps_v")
                for kt in range(NKT):
                    nc.tensor.matmul(
                        psv[:],
                        lhsT=xT_sb[kt][:, ts(t, 128)],
                        rhs=wv_sb[kt][:],
                        start=(kt == 0),
                        stop=(kt == NKT - 1),
                    )
                vv = v_sb[t].rearrange("p (h c) -> p h c", h=4)
                nc.vector.tensor_copy(
                    vv[:, :, 0:64], psv.rearrange("p (h c) -> p h c", h=4)[:]
                )

            def pv_step(mp, j, pvs, e_tiles):
                """PV accumulation for both heads of pair mp, query tile j.
                Each matmul targets one 128-col range (uniform has_written)."""
                for hp in range(2):
                    h = 2 * mp + hp
                    if j % 4 == 0:
                        pvs[hp] = pv_ps("ps_pv")
                    pv = pvs[hp]
                    col = 128 * (j % 4)
                    k2s = [k2 for k2 in (j - 2, j - 1, j) if k2 >= 0]
                    for idx, k2 in enumerate(k2s):
                        nc.tensor.matmul(
                            pv[:, ds(col, 128)],
                            lhsT=v_sb[k2][:, ds(65 * h, 65)],
                            rhs=e_tiles[k2][:, ds(384 * hp + 128 * (j - k2), 128)],
                            start=(idx == 0),
                            stop=(idx == len(k2s) - 1),
                        )

            norm_mode = os.environ.get("ATTN_NORM", "gps")

            def norm_front(pvs):
                """denominator extraction (both heads packed along the free dim
                of partition 0) + reciprocal, copies split across ACT/DVE.
                Returns an SBUF tile holding the reciprocals broadcast (or ready
                to broadcast) across partitions — a DVE mul may read at most
                one PSUM operand, so the broadcast must land in SBUF."""
                d2 = smallp.tile([1, 2 * QB], F32, tag="d", name="d2")
                nc.scalar.copy(d2[:, 0:QB], pvs[0][64:65, :])
                nc.vector.tensor_copy(d2[:, QB : 2 * QB], pvs[1][64:65, :])
                r2 = smallp.tile([1, 2 * QB], F32, tag="r", name="r2")
                nc.vector.reciprocal_approx_fast(r2[:], d2[:])
                if norm_mode == "gps":
                    # custom GpSimd partition-broadcast straight into SBUF
                    rb = smallp.tile([64, 2 * QB], F32, tag="rbb", name="rb_bc")
                    nc.gpsimd.partition_broadcast(rb[:], r2[:], channels=64)
                    return rb
                # DRAM-bounce partition broadcast (SBUF APs can't step-0)
                r_dr = dramp.tile([1, 2 * QB], F32, tag="rdr", name="r_dr")
                nc.gpsimd.dma_start(r_dr[:], r2[:])
                rb = smallp.tile([64, 2 * QB], F32, tag="rbb", name="rb_bc")
                nc.gpsimd.dma_start(rb[:], r_dr[:].to_broadcast([64, 2 * QB]))
                return rb

            def norm_back(mp, b, pvs, rb):
                """normalize multiplies (pv psum x reciprocal-broadcast SBUF),
                issued one kt later so the PE never waits on the reciprocal
                chain. A DVE mul may read at most one PSUM operand, hence the
                SBUF-resident broadcast."""
                for hp in range(2):
                    nc.vector.tensor_mul(
                        aT_sb[mp][ds(64 * hp, 64), ds(QB * b, QB)],
                        pvs[hp][0:64, :],
                        rb[0:64, ds(QB * hp, QB)],
                    )

            def outproj_tile(t):
                ysb = yp.tile([128, C], BF, tag="y", name="ysb")
                for n in range(2):
                    psy = proj_ps("ps_y")
                    for kj in range(2):
                        nc.tensor.matmul(
                            psy[:],
                            lhsT=aT_sb[kj][:, ts(t, 128)],
                            rhs=wo_sb[kj][:, ts(n, 512)],
                            start=(kj == 0),
                            stop=(kj == 1),
                        )
                    if n == 0:
                        nc.scalar.copy(ysb[:, ts(n, 512)], psy[:])
                    else:
                        nc.vector.tensor_copy(ysb[:, ts(n, 512)], psy[:])
                nc.sync.dma_start(y_d[ts(t, 128), :], ysb[:])

            def scores_exp_mask(mp, kt, e_tiles):
                nkt = 128 * min(3, NT - kt)
                sc = psp.tile([128, 1024], F32, tag="sc", bufs=1, name="ps_sc")
                for half in range(2):
                    rows = slice(64 * half, 64 * half + 64)
                    nc.tensor.matmul(
                        sc[:, ds(512 * half, nkt)],
                        lhsT=kT_sb[mp][rows, ts(kt, 128)],
                        rhs=qT_sb[mp][rows, ds(128 * kt, nkt)],
                        start=True,
                        stop=True,
                    )
                E = ep.tile([128, 768], BF, tag="E", name="E")
                scv = sc.rearrange("p (b x) -> p b x", b=2)
                Ev = E.rearrange("p (b x) -> p b x", b=2)
                nc.scalar.activation(
                    Ev[:, :, 0:nkt],
                    scv[:, :, 0:nkt],
                    mybir.ActivationFunctionType.Exp,
                    scale=SCALE,
                )
                # fused mask: diag (cols 0:128) + strict (cols 256:384)
                # of both heads in one op via stepped view
                if kt <= NT - 3:
                    Em = E.rearrange("p (b s x) -> p b s x", b=2, s=3)[:, :, ::2, :]
                    nc.vector.tensor_mul(Em[:], Em[:], maskf_v[:])
                else:
                    nc.vector.tensor_mul(
                        Ev[:, :, 0:128], Ev[:, :, 0:128], maskd_v[:]
                    )
                e_tiles.append(E)

            # ---- mp0 loop: scores + normalize + v/m1 fillers + PV ----
            # In-kt PE order: scores, bc (deps met a kt ago), then v+m1 filler
            # runs while the DVE drains the previous block's normalize muls, so
            # the pv ring slot is free by the time pv_step reallocates it.
            pending = None  # (mp, b, pvs, r2b) awaiting norm_back
            e0 = []
            pvs0 = [None, None]
            for kt in range(NT):
                scores_exp_mask(0, kt, e0)
                if pending is not None:
                    norm_back(*pending)
                    pending = None
                v_tile(kt)
                if kt < 14:
                    m1_chunk(kt)
                pv_step(0, kt, pvs0, e0)
                if kt % 4 == 3:
                    r2b = norm_front(pvs0)
                    pending = (0, kt // 4, list(pvs0), r2b)

            # ---- mp1 loop: scores + normalize + outproj fillers + PV ----
            e1 = []
            pvs1 = [None, None]
            op_t = 0  # next outproj tile to issue
            for kt in range(NT):
                scores_exp_mask(1, kt, e1)
                if pending is not None:
                    norm_back(*pending)
                    pending = None
                if kt < 2:
                    m1_chunk(14 + kt)  # the (k,3) group slides into mp1
                # aT[1] block b is normalized during kt=4b+4; aT[0] fully done.
                lim = min(NT, max(0, 4 * ((kt - 1) // 4)))
                issued = 0
                while op_t < lim and issued < 2:
                    outproj_tile(op_t)
                    op_t += 1
                    issued += 1
                pv_step(1, kt, pvs1, e1)
                if kt % 4 == 3:
                    r2b = norm_front(pvs1)
                    pending = (1, kt // 4, list(pvs1), r2b)

            # tail: last normalize + remaining outproj tiles
            if pending is not None:
                norm_back(*pending)
                pending = None
            while op_t < NT:
                outproj_tile(op_t)
                op_t += 1

    nc.compile()
    return nc


def make_masks():
    one = np.ones((128, 128), np.float32)
    maskd = np.triu(one)          # keep iff i >= kk  (diag tile)
    masks_ = np.tril(one, -1)     # keep iff i <  kk  (strict tile)
    md2 = np.concatenate([maskd, maskd], axis=1).astype(BF16)
    ms2 = np.concatenate([masks_, masks_], axis=1).astype(BF16)
    return md2, ms2


def make_in_maps(x, wq, wk, wv, wo):
    x = np.asarray(x, np.float32)
    wq, wk, wv, wo = (np.asarray(a, np.float32) for a in (wq, wk, wv, wo))
    md2, ms2 = make_masks()
    md, ms = md2[:, :128], ms2[:, :128]
    # [maskd|masks|maskd|masks]: matches (head, subtile, col) iteration of the
    # fused mask op; cols 0:256 also serve the per-head diag/strict views
    mask_all = np.hstack([md, ms, md, ms])  # [128, 512]

    def tile_rows(a):  # [1024, W] -> [128, 8*W] (row-blocks side by side)
        return np.hstack([a[i * 128 : (i + 1) * 128] for i in range(a.shape[0] // 128)])

    in_maps = []
    for c in range(8):
        b, g = divmod(c, 4)
        sl = slice(g * CL, (g + 1) * CL)
        xTt = tile_rows(np.ascontiguousarray(x[b].T).astype(BF16))
        wt = np.hstack(
            [
                tile_rows(np.ascontiguousarray(w[sl, :].T).astype(BF16))
                for w in (wq, wk, wv)
            ]
        )
        wot = tile_rows(np.ascontiguousarray(wo[:, sl].T).astype(BF16))
        in_maps.append(
            {"xTt": xTt, "wt": wt, "wot": wot, "maskt": mask_all}
        )
    return in_maps


_PROG = None


def _get_prog():
    global _PROG
    if _PROG is None:
        _PROG = build_program()
    return _PROG


def kernel(x, wq, wk, wv, wo, _trace=False, _tmpdir=None):
    from concourse.bass_utils import run_bass_kernel_spmd

    nc = _get_prog()
    in_maps = make_in_maps(x, wq, wk, wv, wo)
    res = run_bass_kernel_spmd(
        nc, in_maps, core_ids=list(range(8)), trace=_trace, tmpdir=_tmpdir
    )
    y = np.zeros((B, T, C), np.float32)
    for c in range(8):
        b = c // 4
        y[b] += res.results[c]["y"].astype(np.float32)
    if _trace:
        kernel._last_results = res
    return y
